# revision 17
# baseline (speedup 1.0000x reference)
"""2-layer GAT on 8 NeuronCores (Trainium2, Bass/Tile).

Strategy (dst-sharded graph parallel):
  - Each core owns 12500 dst nodes (padded to 12544 = 98*128).
  - Layer tables: row = [feats(64) | a_s(heads)] f32, one row per node,
    quarter-major interleaved so the AllGather runs in 4 chunks per
    layer, each overlapped with the producing compute.
  - Edges grouped by dst owner; SELF-LOOPS EXCLUDED from the gather
    slots: each tile loads its own 128 rows with one direct (HWDGE)
    DMA instead of 128 indirect-gather slots -- the indirect gathers
    on the GpSimd Q7 engine (~1.17us per 128-row call) are the span
    bottleneck, so every removed slot column counts.
  - Per 128-node tile, K slot columns (degree-sorted tiles => low
    padding); source rows fetched with one indirect DMA per slot
    column; gather pool triple-buffered so Q7 streams calls
    back-to-back while DVE consumes earlier tiles.
  - Segment softmax without max-subtraction (logits are O(10), exact
    same alpha ratios), weighted sum on DVE, ELU composed from
    min/exp/max, layer-1 projection fused per tile on the PE.
"""
import sys

sys.path.insert(0, "/opt/trn_rl_repo")

import numpy as np

import concourse.bass as bass
import concourse.mybir as mybir
import concourse.tile as tile
from concourse import bacc
from concourse.masks import make_identity
from concourse.bass_utils import run_bass_kernel_spmd

P = 128
NCORES = 8
NEG_SLOPE = 0.2
PAD_AS = -1e30
# AllGather chunk row boundaries (local rows, per layer); last chunk kept
# small so the producing loop's tail exposes minimal AG latency.
CHUNK_B = [0, 3200, 6400, 9600, 11648, 12544]


class Cfg:
    def __init__(self, n=100000, e=1600000, fin=128, heads=8, ch=8, out=64):
        self.N, self.E, self.IN, self.H, self.C, self.OUT = n, e, fin, heads, ch, out
        self.n_per = n // NCORES                      # owned real nodes
        self.blocks = (self.n_per + P - 1) // P       # tiles per device
        self.n_pad = self.blocks * P                  # padded nodes per device
        self.rows = NCORES * self.n_pad               # table rows
        self.w0cols = out + heads + heads             # feats | a_s | a_d
        self.t0cols = out + heads                     # table row cols layer0
        self.t1cols = out + 1                         # table row cols layer1
        assert CHUNK_B[-1] == self.n_pad


def _prep(cfg, x, edge_index):
    """Host-side sharding: permutation, per-device tiles, gather indices.

    Self-loops are NOT added to the edge slots; the kernel adds the own-row
    contribution via a direct per-tile DMA column.
    """
    N, n_per, blocks, n_pad = cfg.N, cfg.n_per, cfg.blocks, cfg.n_pad
    src = np.asarray(edge_index[0], dtype=np.int64)
    dst = np.asarray(edge_index[1], dtype=np.int64)

    owner = dst // n_per
    # degree INCLUDING self-loop so tiles sort the same as the softmax size
    deg = np.bincount(dst, minlength=N)

    # chunk-major interleave: local row r in chunk ci maps to global row
    # NCORES*B[ci] + d*(B[ci+1]-B[ci]) + (r - B[ci])
    def row_of(cpos, d):
        ci = np.searchsorted(np.asarray(CHUNK_B), cpos, side="right") - 1
        b0 = np.asarray(CHUNK_B)[ci]
        b1 = np.asarray(CHUNK_B)[ci + 1]
        return NCORES * b0 + d * (b1 - b0) + (cpos - b0)

    perm_l = []          # perm_l[d] = original node ids in canonical order
    xts = []             # per-device x^T [IN, n_pad]
    idx_cols = []        # per-device int32 [P, C] gather indices
    kmax_l = []          # per-device list of K per tile
    g_of = np.empty(N, dtype=np.int64)   # original node -> global table row
    for d in range(NCORES):
        lo, hi = d * n_per, (d + 1) * n_per
        nodes = np.arange(lo, hi)
        order = np.argsort(-deg[lo:hi], kind="stable")
        canon = nodes[order]                       # canonical order, len n_per
        perm_l.append(canon)
        cpos = np.arange(n_per)
        g_of[canon] = row_of(cpos, d)

    # device-0 pad row (canonical pos n_per of device 0, a_s = -1e30)
    dummy = int(row_of(np.int64(cfg.n_per), 0))

    for d in range(NCORES):
        lo, hi = d * n_per, (d + 1) * n_per
        m = owner == d
        es, ed = src[m], dst[m]
        pos = np.empty(n_per, dtype=np.int64)
        pos[(perm_l[d] - lo)] = np.arange(n_per)
        ep = pos[ed - lo]                          # canonical pos of each edge's dst
        order = np.argsort(ep, kind="stable")
        es, ep = es[order], ep[order]
        counts = np.bincount(ep, minlength=n_pad)
        starts = np.concatenate([[0], np.cumsum(counts)])
        kmax = []
        cols = []
        for t in range(blocks):
            c = counts[t * P:(t + 1) * P]
            K = max(1, int(c.max()))
            kmax.append(K)
            tilecols = np.full((P, K), dummy, dtype=np.int32)
            for p in range(P):
                node = t * P + p
                s0, s1 = starts[node], starts[node + 1]
                if s1 > s0:
                    tilecols[p, :s1 - s0] = g_of[es[s0:s1]]
            cols.append(tilecols)
        idx_cols.append(np.concatenate(cols, axis=1))  # [P, sum K]
        kmax_l.append(kmax)
        xt = np.zeros((cfg.IN, n_pad), dtype=np.float32)
        xt[:, :n_per] = x[perm_l[d]].T
        xts.append(xt)
    return perm_l, xts, idx_cols, kmax_l, dummy


def _build(cfg, kmax, ncols):
    H, C = cfg.H, cfg.C
    n_pad, blocks, rows = cfg.n_pad, cfg.blocks, cfg.rows
    T0, T1 = cfg.t0cols, cfg.t1cols
    f32 = mybir.dt.float32
    groups = [list(range(NCORES))]
    NCHUNK = len(CHUNK_B) - 1

    nc = bacc.Bacc(num_devices=NCORES)
    xt = nc.declare_dram_parameter("xt", [cfg.IN, n_pad], f32, isOutput=False)
    idx = nc.declare_dram_parameter("idx", [P, ncols], mybir.dt.int32, isOutput=False)
    w0 = nc.declare_dram_parameter("w0", [cfg.IN, cfg.w0cols], f32, isOutput=False)
    w1 = nc.declare_dram_parameter("w1", [P, cfg.OUT + 2], f32, isOutput=False)
    bias = nc.declare_dram_parameter("bias", [2, cfg.OUT], f32, isOutput=False)
    out_d = nc.declare_dram_parameter("out", [n_pad, cfg.OUT], f32, isOutput=True)

    ltab0 = nc.dram_tensor("ltab0", [n_pad, T0], f32)
    ltab1 = nc.dram_tensor("ltab1", [n_pad, T1], f32)
    tab0 = nc.dram_tensor("tab0", [rows, T0], f32, addr_space="Shared")
    tab1 = nc.dram_tensor("tab1", [rows, T1], f32, addr_space="Shared")

    # chunk boundaries: AG chunk i fires once tiles covering rows
    # [0, CHUNK_B[i+1]) are written (boundaries need not be tile-aligned)
    chunk_end = [(CHUNK_B[i + 1] + P - 1) // P - 1 for i in range(NCHUNK)]
    chunk_end[-1] = blocks - 1

    def ag(table_l, table_g, width, ci):
        b0, b1 = CHUNK_B[ci], CHUNK_B[ci + 1]
        nc.gpsimd.collective_compute(
            "AllGather", mybir.AluOpType.bypass,
            replica_groups=groups,
            ins=[table_l[b0:b1, :]],
            outs=[table_g[NCORES * b0:NCORES * b1, :]])

    with tile.TileContext(nc) as tc:
        with (
            tc.tile_pool(name="persist", bufs=1) as pp,
            tc.tile_pool(name="work", bufs=6) as wp,
            tc.tile_pool(name="gat", bufs=4) as gq,
            tc.tile_pool(name="gsc", bufs=2) as gp,
            tc.tile_pool(name="ps", bufs=4, space="PSUM") as psp,
            tc.tile_pool(name="ps2", bufs=2, space="PSUM") as psp2,
        ):
            # ---- constants ----
            w0t = pp.tile([cfg.IN, cfg.w0cols], f32)
            nc.sync.dma_start(out=w0t[:], in_=w0[:])
            w1t = pp.tile([P, cfg.OUT + 2], f32)
            nc.sync.dma_start(out=w1t[:], in_=w1[:])
            b0t = pp.tile([P, cfg.OUT], f32)
            nc.sync.dma_start(out=b0t[:], in_=bias[0:1, :].to_broadcast([P, cfg.OUT]))
            b1t = pp.tile([P, cfg.OUT], f32)
            nc.sync.dma_start(out=b1t[:], in_=bias[1:2, :].to_broadcast([P, cfg.OUT]))
            idxt = pp.tile([P, ncols], mybir.dt.int32)
            nc.sync.dma_start(out=idxt[:], in_=idx[:])
            a_d0 = pp.tile([P, blocks * H], f32)
            a_d1 = pp.tile([P, blocks], f32)
            ident = pp.tile([P, P], f32)
            make_identity(nc, ident[:])
            pad_as = pp.tile([P, H], f32)
            nc.vector.memset(pad_as[:], PAD_AS)

            # ---- P1: xW matmuls -> ltab0 + a_d ----
            # whole x^T resident in SBUF (one bulk DMA, ~50KB/partition):
            # removes every per-tile load from P1's dependency chain
            xtt = pp.tile([cfg.IN, n_pad], f32)
            nc.sync.dma_start(out=xtt[:], in_=xt[:])
            ci = 0
            for t in range(blocks):
                ps = psp.tile([P, cfg.w0cols], f32, tag="mm0")
                nc.tensor.matmul(out=ps[:], lhsT=xtt[:, t * P:(t + 1) * P],
                                 rhs=w0t[:], start=True, stop=True)
                row = wp.tile([P, T0], f32, tag="row0")
                nc.scalar.copy(out=row[:], in_=ps[:, 0:T0])
                nc.vector.tensor_copy(out=a_d0[:, t * H:(t + 1) * H],
                                      in_=ps[:, T0:T0 + H])
                nc.sync.dma_start(out=ltab0[t * P:(t + 1) * P, :], in_=row[:])
                if t == blocks - 1:
                    # pad rows: a_s = -1e30 (rows n_per..n_pad-1) before last AG
                    npad_rows = n_pad - cfg.n_per
                    if npad_rows > 0:
                        nc.sync.dma_start(
                            out=ltab0[cfg.n_per:n_pad, cfg.OUT:cfg.OUT + H],
                            in_=pad_as[0:npad_rows, :])
                if ci < NCHUNK and t == chunk_end[ci]:
                    ag(ltab0, tab0, T0, ci)
                    ci += 1

            # ---- L0 edge phase + fused L1 projection ----
            ci = 0
            col = 0
            for t in range(blocks):
                K = kmax[t]
                KT = K + 1  # + self column
                g = gq.tile([P, KT * T0], f32, tag="g0")
                for k in range(K):
                    nc.gpsimd.indirect_dma_start(
                        out=g[:, k * T0:(k + 1) * T0],
                        out_offset=None,
                        in_=tab0[:, :],
                        in_offset=bass.IndirectOffsetOnAxis(
                            ap=idxt[:, col + k:col + k + 1], axis=0))
                col += K
                # self column: own rows, direct HWDGE load
                nc.sync.dma_start(out=g[:, K * T0:K * T0 + T0],
                                  in_=ltab0[t * P:(t + 1) * P, :])
                gv = g[:].rearrange("p (k w) -> p k w", w=T0)
                # e[p,h,k] = a_s[src] + a_d[dst]
                e = gp.tile([P, H * KT], f32, tag="e")
                ev = e[:].rearrange("p (h k) -> p h k", k=KT)
                asg = gv[:, :, cfg.OUT:T0].rearrange("p k h -> p h k")
                nc.vector.tensor_tensor(
                    out=ev, in0=asg,
                    in1=a_d0[:, t * H:(t + 1) * H].to_broadcast([P, H, KT]),
                    op=mybir.AluOpType.add)
                scr = gp.tile([P, H * KT], f32, tag="scr")
                nc.vector.tensor_scalar(
                    out=scr[:], in0=e[:], scalar1=NEG_SLOPE, scalar2=-88.0,
                    op0=mybir.AluOpType.mult, op1=mybir.AluOpType.max)
                nc.vector.tensor_tensor(out=e[:], in0=e[:], in1=scr[:],
                                        op=mybir.AluOpType.max)
                nc.scalar.activation(out=e[:], in_=e[:],
                                     func=mybir.ActivationFunctionType.Exp)
                den = gp.tile([P, H], f32, tag="den")
                nc.vector.tensor_reduce(out=den[:], in_=ev,
                                        axis=mybir.AxisListType.X,
                                        op=mybir.AluOpType.add)
                nc.vector.reciprocal(out=den[:], in_=den[:])
                # unnormalized weighted sum, normalize after the reduce
                prod = gp.tile([P, cfg.OUT * KT], f32, tag="prod")
                pv = prod[:].rearrange("p (h c k) -> p h c k", c=C, k=KT)
                al_b = bass.AP(ev.tensor, ev.offset,
                               [ev.ap[0], ev.ap[1], [0, C], ev.ap[2]])
                nc.vector.tensor_tensor(
                    out=pv,
                    in0=al_b,
                    in1=gv[:, :, 0:cfg.OUT].rearrange(
                        "p k (h c) -> p h c k", c=C),
                    op=mybir.AluOpType.mult)
                hfeat = gp.tile([P, cfg.OUT], f32, tag="hfeat")
                hv = hfeat[:].rearrange("p (h c) -> p h c", c=C)
                nc.vector.tensor_reduce(
                    out=hfeat[:], in_=pv, axis=mybir.AxisListType.X,
                    op=mybir.AluOpType.add)
                nc.vector.tensor_tensor(
                    out=hv, in0=hv, in1=den[:].to_broadcast([P, H, C]),
                    op=mybir.AluOpType.mult)
                nc.vector.tensor_add(out=hfeat[:], in0=hfeat[:], in1=b0t[:])
                # ELU: h = max(x,0) + exp(min(x,0)) - 1
                tmn = gp.tile([P, cfg.OUT], f32, tag="tmn")
                nc.vector.tensor_scalar_min(out=tmn[:], in0=hfeat[:], scalar1=0.0)
                nc.scalar.activation(out=tmn[:], in_=tmn[:],
                                     func=mybir.ActivationFunctionType.Exp)
                nc.vector.tensor_scalar_max(out=hfeat[:], in0=hfeat[:], scalar1=0.0)
                nc.vector.tensor_tensor(out=hfeat[:], in0=hfeat[:], in1=tmn[:],
                                        op=mybir.AluOpType.add)
                nc.vector.tensor_scalar_add(out=hfeat[:], in0=hfeat[:], scalar1=-1.0)
                # L1 projection: rows of ltab1 = [h @ W1 | h @ w_src1]; a_d1 kept
                pst = psp2.tile([P, P], f32, tag="tr")
                nc.tensor.transpose(out=pst[:cfg.OUT, :], in_=hfeat[:],
                                    identity=ident[:])
                ht = wp.tile([cfg.OUT, P], f32, tag="ht")
                nc.scalar.copy(out=ht[:], in_=pst[:cfg.OUT, :])
                ps1 = psp2.tile([P, cfg.OUT + 2], f32, tag="mm1")
                nc.tensor.matmul(out=ps1[:], lhsT=ht[:],
                                 rhs=w1t[:cfg.OUT, :], start=True, stop=True)
                row1 = wp.tile([P, T1], f32, tag="row1")
                nc.scalar.copy(out=row1[:], in_=ps1[:, 0:T1])
                nc.vector.tensor_copy(out=a_d1[:, t:t + 1],
                                      in_=ps1[:, T1:T1 + 1])
                nc.sync.dma_start(out=ltab1[t * P:(t + 1) * P, :], in_=row1[:])
                if t == blocks - 1:
                    npad_rows = n_pad - cfg.n_per
                    if npad_rows > 0:
                        nc.sync.dma_start(
                            out=ltab1[cfg.n_per:n_pad, cfg.OUT:cfg.OUT + 1],
                            in_=pad_as[0:npad_rows, 0:1])
                if ci < NCHUNK and t == chunk_end[ci]:
                    ag(ltab1, tab1, T1, ci)
                    ci += 1

            # ---- L1 edge phase ----
            col = 0
            for t in range(blocks):
                K = kmax[t]
                KT = K + 1
                g = gq.tile([P, KT * T1], f32, tag="g1")
                for k in range(K):
                    nc.gpsimd.indirect_dma_start(
                        out=g[:, k * T1:(k + 1) * T1],
                        out_offset=None,
                        in_=tab1[:, :],
                        in_offset=bass.IndirectOffsetOnAxis(
                            ap=idxt[:, col + k:col + k + 1], axis=0))
                col += K
                nc.sync.dma_start(out=g[:, K * T1:K * T1 + T1],
                                  in_=ltab1[t * P:(t + 1) * P, :])
                gv = g[:].rearrange("p (k w) -> p k w", w=T1)
                e = gp.tile([P, KT], f32, tag="e1")
                asg1 = gv[:, :, cfg.OUT:T1].rearrange("p k w -> p (k w)")
                nc.vector.tensor_tensor(
                    out=e[:], in0=asg1,
                    in1=a_d1[:, t:t + 1].to_broadcast([P, KT]),
                    op=mybir.AluOpType.add)
                scr1 = gp.tile([P, KT], f32, tag="scr1")
                nc.vector.tensor_scalar(
                    out=scr1[:], in0=e[:], scalar1=NEG_SLOPE, scalar2=-88.0,
                    op0=mybir.AluOpType.mult, op1=mybir.AluOpType.max)
                nc.vector.tensor_tensor(out=e[:], in0=e[:], in1=scr1[:],
                                        op=mybir.AluOpType.max)
                nc.scalar.activation(out=e[:], in_=e[:],
                                     func=mybir.ActivationFunctionType.Exp)
                den = gp.tile([P, 1], f32, tag="den1")
                nc.vector.tensor_reduce(out=den[:], in_=e[:],
                                        axis=mybir.AxisListType.X,
                                        op=mybir.AluOpType.add)
                nc.vector.reciprocal(out=den[:], in_=den[:])
                prod = gp.tile([P, cfg.OUT * KT], f32, tag="prod1")
                pv = prod[:].rearrange("p (c k) -> p c k", k=KT)
                e_ap = e[:]
                al_b = bass.AP(e_ap.tensor, e_ap.offset,
                               [e_ap.ap[0], [0, cfg.OUT], e_ap.ap[1]])
                nc.vector.tensor_tensor(
                    out=pv,
                    in0=al_b,
                    in1=gv[:, :, 0:cfg.OUT].rearrange("p k c -> p c k"),
                    op=mybir.AluOpType.mult)
                of = gp.tile([P, cfg.OUT], f32, tag="of")
                nc.vector.tensor_reduce(out=of[:], in_=pv,
                                        axis=mybir.AxisListType.X,
                                        op=mybir.AluOpType.add)
                nc.vector.tensor_tensor(
                    out=of[:], in0=of[:],
                    in1=den[:].to_broadcast([P, cfg.OUT]),
                    op=mybir.AluOpType.mult)
                nc.vector.tensor_add(out=of[:], in0=of[:], in1=b1t[:])
                tmn = gp.tile([P, cfg.OUT], f32, tag="tmn1")
                nc.vector.tensor_scalar_min(out=tmn[:], in0=of[:], scalar1=0.0)
                nc.scalar.activation(out=tmn[:], in_=tmn[:],
                                     func=mybir.ActivationFunctionType.Exp)
                nc.vector.tensor_scalar_max(out=of[:], in0=of[:], scalar1=0.0)
                nc.vector.tensor_tensor(out=of[:], in0=of[:], in1=tmn[:],
                                        op=mybir.AluOpType.add)
                nc.vector.tensor_scalar_add(out=of[:], in0=of[:], scalar1=-1.0)
                nc.sync.dma_start(out=out_d[t * P:(t + 1) * P, :], in_=of[:])
    nc.finalize()
    return nc


def kernel(x, edge_index, W0, att_src0, att_dst0, b0, W1, att_src1, att_dst1, b1,
           _cfg=None):
    cfg = _cfg or Cfg()
    x = np.asarray(x, dtype=np.float32)
    W0 = np.asarray(W0, np.float32)
    W1 = np.asarray(W1, np.float32)
    att_src0 = np.asarray(att_src0, np.float32)
    att_dst0 = np.asarray(att_dst0, np.float32)
    att_src1 = np.asarray(att_src1, np.float32)
    att_dst1 = np.asarray(att_dst1, np.float32)
    b0 = np.asarray(b0, np.float32)
    b1 = np.asarray(b1, np.float32)

    assert cfg.n_pad > cfg.n_per, "need at least one pad row for dummy slots"
    perm_l, xts, idx_cols, kmax_l, dummy = _prep(cfg, x, edge_index)
    # unify per-tile K across devices (SPMD: one program)
    blocks = cfg.blocks
    kmax = [max(kmax_l[d][t] for d in range(NCORES)) for t in range(blocks)]
    ncols = int(np.sum(kmax))
    idx_u = []
    for d in range(NCORES):
        buf = np.full((P, ncols), dummy, dtype=np.int32)
        c_s = 0
        c_d = 0
        for t in range(blocks):
            kd = kmax_l[d][t]
            buf[:, c_d:c_d + kd] = idx_cols[d][:, c_s:c_s + kd]
            c_s += kd
            c_d += kmax[t]
        idx_u.append(buf)

    # weights: A blockdiag for layer0 attention
    H, C = cfg.H, cfg.C
    A_src = np.zeros((H * C, H), np.float32)
    A_dst = np.zeros((H * C, H), np.float32)
    for h in range(H):
        A_src[h * C:(h + 1) * C, h] = att_src0[h]
        A_dst[h * C:(h + 1) * C, h] = att_dst0[h]
    w0cat = np.concatenate([W0, W0 @ A_src, W0 @ A_dst], axis=1)  # [IN, 80]
    w1cat = np.zeros((P, cfg.OUT + 2), np.float32)
    w1cat[:cfg.OUT] = np.concatenate(
        [W1, W1 @ att_src1[0][:, None], W1 @ att_dst1[0][:, None]], axis=1)
    biases = np.stack([b0.reshape(-1), b1.reshape(-1)])

    nc = _build(cfg, kmax, ncols)
    in_maps = []
    for d in range(NCORES):
        in_maps.append({
            "xt": xts[d],
            "idx": idx_u[d],
            "w0": w0cat,
            "w1": w1cat,
            "bias": biases,
        })
    res = run_bass_kernel_spmd(nc, in_maps, core_ids=list(range(NCORES)))
    out = np.empty((cfg.N, cfg.OUT), np.float32)
    for d in range(NCORES):
        out[perm_l[d]] = res.results[d]["out"][:cfg.n_per]
    return out
